# revision 44
# baseline (speedup 1.0000x reference)
"""Trainium2 Bass kernel: decoder GQA attention with RoPE, tensor-parallel over 8 NeuronCores.

Sharding: core c = (h, g) with h = c//4, g = c%4 handles the 4 query heads of
GQA group g (heads 4g..4g+3, which share KV head g) for the 2 batches
{2h, 2h+1}.  Compared with 2-heads x 4-batches per core this removes the
duplicated K/V projections entirely (48 instead of 64 projection matmuls per
token chunk) without any extra collective, halves the x DMA, and widens the
output-projection matmuls to 512 moving columns.  All matmul operands are
bf16 (same PE rate as fp32r, half the DMA/SBUF traffic); PSUM and softmax
denominators stay fp32.  Per core:
  - Constants (RoPE tables, causal masks, Wo^T, bias) are DMA'd at kernel
    start so the attention phase never waits on them; Wo^T chunks are
    interleaved with the projection units to stay off the x-stream's critical
    path.
  - QKV projection of the core's 2 batches against its [C, 768] weight slice,
    emitted output-major (q01 | q23 | kv accumulation groups) so each PSUM
    ring slot drains while the next group computes; RoPE on the fly; q/k/v
    stay SBUF-resident.
  - Flash-style causal attention with transposed scores (sT[k,q]) in
    [128,1024] PSUM tiles, exp batched per 1024 cols on the Scalar engine,
    software-pipelined so PV matmuls of the previous key-chunk fill the PE
    while the current chunk exponentiates.  Scores AND PV matmuls in the
    fully-masked region of diagonal tiles are skipped (partial moving dims;
    the PV split carries per-column-segment stop flags).  The softmax
    denominator is a bf16 fold-tree on the Vector engine plus one ones-matmul
    per query chunk; normalization uses the fast approximate reciprocal.
  - One 8-core AllToAll per local batch index reshards the attention output
    head->token in 256-token chunks (cores 0-3 contribute batch lb, cores
    4-7 batch 2+lb, so every core ends with both batches' full channels for
    its token slice and the output projection keeps 512 moving columns);
    wo(lb=0) is emission-interleaved with attn(lb=1) so its matmuls fill the
    exp-pipeline bubbles and hide the collective; bias is fused into the
    Scalar PSUM->SBUF copy; the host transposes at assembly.
"""

import os
import sys

for _p in ("/opt/trn_rl_repo",):
    if _p not in sys.path:
        sys.path.insert(0, _p)

import numpy as np
from ml_dtypes import bfloat16

import concourse.bacc as bacc
import concourse.mybir as mybir
import concourse.tile as tile
from concourse.bass_utils import run_bass_kernel_spmd

F32 = mybir.dt.float32
BF16 = mybir.dt.bfloat16
AX = mybir.AluOpType

B, T, C = 4, 2048, 2048
N_HEAD, N_KV = 16, 4
HD = C // N_HEAD            # 128
NCORES = 8
GROUPS = [list(range(NCORES))]
HPC = 4                     # q heads per core (one full GQA group)
BL = 2                      # local batches per core
SCALE = 1.0 / float(np.sqrt(HD))
TQ = 512                    # query-chunk (psum free dim)
NQC = T // TQ               # 4 query chunks per (lb, head)
CCH = C // 128              # 16 contraction chunks
TW = T // NCORES            # 256 tokens per (core, batch) in the wo shard

_CACHE = {}


def _build():
    """Build + compile the per-core Bass graph (same graph for every core)."""
    nc = bacc.Bacc(
        "TRN2",
        target_bir_lowering=False,
        debug=False,
        enable_asserts=False,
        num_devices=NCORES,
    )

    xt_d = nc.dram_tensor("xt", [BL, C, T], BF16, kind="ExternalInput")
    wqkv_d = nc.dram_tensor("wqkv", [C, 768], BF16, kind="ExternalInput")
    wot_d = nc.dram_tensor("wot", [C, C], BF16, kind="ExternalInput")
    cc_d = nc.dram_tensor("ropec", [128, T], BF16, kind="ExternalInput")
    ss_d = nc.dram_tensor("ropes", [128, T], BF16, kind="ExternalInput")
    mask_d = nc.dram_tensor("masks", [128, 4 * TQ], BF16, kind="ExternalInput")
    negid_d = nc.dram_tensor("negid", [128, 128], BF16, kind="ExternalInput")
    ones_d = nc.dram_tensor("ones", [128, 128], BF16, kind="ExternalInput")
    ident_d = nc.dram_tensor("ident", [128, 128], BF16, kind="ExternalInput")
    boc_d = nc.dram_tensor("boc", [128, CCH], F32, kind="ExternalInput")
    out_d = nc.dram_tensor("out", [C, BL * 2 * TW], F32, kind="ExternalOutput")

    with tile.TileContext(nc) as tc:
        with tc.tile_pool(name="dram", bufs=1, space="DRAM") as dp:
            # one buffer per (local batch, head-half): halved collectives
            # start earlier and the scheduler's collective-cost model then
            # lets wo matmuls fill attention stalls sooner
            in_bufs = [
                [dp.tile([C, TW], BF16, name=f"in_buf{b}_{h2}") for h2 in range(2)]
                for b in range(BL)
            ]
            out_bufs = [
                [dp.tile([C, TW], BF16, name=f"out_buf{b}_{h2}") for h2 in range(2)]
                for b in range(BL)
            ]
            warm_in = dp.tile([NCORES, 64], BF16, name="warm_in")
            warm_out = dp.tile([NCORES, 64], BF16, name="warm_out")
            with tc.tile_pool(name="res", bufs=1) as rp:
                # constants resident for the whole kernel; DMAs are emitted by
                # phase 1 after the first unit's w/x loads so the first
                # matmuls are not queued behind them
                id_sb = rp.tile([128, 128], BF16, name="id_sb")
                cc_sb = rp.tile([128, T], BF16, name="cc_sb")
                ss_sb = rp.tile([128, T], BF16, name="ss_sb")
                mask_sb = rp.tile([128, 4 * TQ], BF16, name="mask_sb")
                negid_sb = rp.tile([128, 128], BF16, name="negid_sb")
                ones_sb = rp.tile([128, 128], BF16, name="ones_sb")
                boc_sb = rp.tile([128, CCH], F32, name="boc_sb")
                wot_sb = rp.tile([128, CCH * C], BF16, name="wot_sb")

                def emit_consts():
                    nc.sync.dma_start(out=id_sb[:], in_=ident_d.ap())
                    nc.sync.dma_start(out=cc_sb[:], in_=cc_d.ap())
                    nc.sync.dma_start(out=ss_sb[:], in_=ss_d.ap())
                    nc.sync.dma_start(out=mask_sb[:], in_=mask_d.ap())
                    nc.sync.dma_start(out=negid_sb[:], in_=negid_d.ap())
                    nc.sync.dma_start(out=ones_sb[:], in_=ones_d.ap())
                    nc.sync.dma_start(out=boc_sb[:], in_=boc_d.ap())

                kt_all = rp.tile([128, BL * T], BF16, name="kt_all")
                vstd_all = rp.tile([128, BL * T], BF16, name="vstd_all")
                q_all = rp.tile([128, HPC * BL * T], BF16, name="q_all")

                _phase1_qkv(nc, tc, xt_d, wqkv_d, wot_d, id_sb, cc_sb, ss_sb,
                            wot_sb, q_all, kt_all, vstd_all, emit_consts)
                # warm the collective rings after phase 1: the trigger blocks
                # the GpSimd queue until the collective completes, and the
                # phase-1 x-tile DMAs live on that queue
                nc.gpsimd.collective_compute(
                    "AllToAll",
                    AX.bypass,
                    replica_groups=GROUPS,
                    ins=[warm_in.opt()],
                    outs=[warm_out.opt()],
                )
                _phase2_attn_wo(nc, tc, mask_sb, negid_sb, ones_sb, boc_sb,
                                wot_sb, q_all, kt_all, vstd_all, in_bufs,
                                out_bufs, out_d)

    nc.compile()
    return nc


def _phase1_qkv(nc, tc, xt_d, wqkv_d, wot_d, id_sb, cc_sb, ss_sb, wot_sb,
                q_all, kt_all, vstd_all, emit_consts):
    with (
        tc.tile_pool(name="p1c", bufs=1) as p1c,
        tc.tile_pool(name="px", bufs=32) as px,
        tc.tile_pool(name="pt", bufs=3) as pt,
        tc.tile_pool(name="pp", bufs=3, space="PSUM") as pp,
        tc.tile_pool(name="pst", bufs=2, space="PSUM") as pst,
    ):
        # first unit's weights (Sync queue) and x tiles (Scalar queue) issue
        # in parallel so the first ci-major matmul group starts after ~2
        # DMAs.  x tiles stay off the GpSimd queue: the collective triggers
        # live there and block in-order until their collective completes.
        w_sb = p1c.tile([128, CCH * 768], BF16, name="w_sb")
        warm_xts = []
        for ci in range(CCH):
            nc.sync.dma_start(
                out=w_sb[:, ci * 768 : (ci + 1) * 768],
                in_=wqkv_d[ci * 128 : (ci + 1) * 128, :],
            )
            xtile = px.tile([128, TQ], BF16, tag="xt", name="xt")
            nc.scalar.dma_start(
                out=xtile[:], in_=xt_d[0, ci * 128 : (ci + 1) * 128, 0:TQ]
            )
            warm_xts.append(xtile)
        emit_consts()

        def rope(psrc, dst_ap, cs):
            # dst = src*cc + swap_halves(src)*ss   (rotate-half RoPE)
            qs = pt.tile([128, TQ], BF16, tag="qs", name="qs")
            nc.scalar.copy(qs[:], psrc)
            qsw = pt.tile([128, TQ], BF16, tag="qsw", name="qsw")
            nc.scalar.dma_start(out=qsw[0:64, :], in_=qs[64:128, :])
            nc.scalar.dma_start(out=qsw[64:128, :], in_=qs[0:64, :])
            tm1 = pt.tile([128, TQ], BF16, tag="tm1", name="tm1")
            nc.vector.tensor_tensor(tm1[:], qs[:], cc_sb[:, cs], AX.mult)
            tm2 = pt.tile([128, TQ], BF16, tag="tm2", name="tm2")
            nc.vector.tensor_tensor(tm2[:], qsw[:], ss_sb[:, cs], AX.mult)
            nc.vector.tensor_tensor(dst_ap, tm1[:], tm2[:], AX.add)

        unit = 0
        for lb in range(BL):
            for n in range(NQC):
                if lb == 0 and n == 0:
                    xts = warm_xts
                else:
                    xts = []
                    for ci in range(CCH):
                        xtile = px.tile([128, TQ], BF16, tag="xt", name="xt")
                        eng = nc.scalar if ci % 2 == 0 else nc.sync
                        eng.dma_start(
                            out=xtile[:],
                            in_=xt_d[
                                lb, ci * 128 : (ci + 1) * 128, n * TQ : (n + 1) * TQ
                            ],
                        )
                        xts.append(xtile)
                cs = slice(n * TQ, (n + 1) * TQ)
                # output-major accumulation groups: q01 | q23 | kv, so each
                # PSUM ring slot is drained while the next group computes.
                # The first unit is ci-major instead: its matmuls then only
                # wait for w[0]/x[0] rather than the full weight+x load.
                ps = [
                    pp.tile([128, 2 * TQ], F32, tag="proj", name=f"ps{i}")
                    for i in range(3)
                ]
                if unit == 0:
                    for ci in range(CCH):
                        for m in range(6):
                            nc.tensor.matmul(
                                ps[m // 2][:, (m % 2) * TQ : (m % 2 + 1) * TQ],
                                w_sb[:, ci * 768 + m * 128 : ci * 768 + (m + 1) * 128],
                                xts[ci][:],
                                start=(ci == 0),
                                stop=(ci == CCH - 1),
                            )
                for grp in range(3):
                    if unit > 0:
                        for ci in range(CCH):
                            for half in range(2):
                                m = 2 * grp + half
                                nc.tensor.matmul(
                                    ps[grp][:, half * TQ : (half + 1) * TQ],
                                    w_sb[:, ci * 768 + m * 128 : ci * 768 + (m + 1) * 128],
                                    xts[ci][:],
                                    start=(ci == 0),
                                    stop=(ci == CCH - 1),
                                )
                    if grp < 2:
                        for half in range(2):
                            hl = 2 * grp + half
                            rope(ps[grp][:, half * TQ : (half + 1) * TQ],
                                 q_all[:, (hl * BL + lb) * T + n * TQ :
                                       (hl * BL + lb) * T + (n + 1) * TQ], cs)
                    else:
                        rope(ps[grp][:, 0:TQ],
                             kt_all[:, lb * T + n * TQ : lb * T + (n + 1) * TQ],
                             cs)
                        vt = pt.tile([128, TQ], BF16, tag="vt", name="vt")
                        nc.scalar.copy(vt[:], ps[grp][:, TQ : 2 * TQ])
                        ptr = pst.tile([128, TQ], BF16, tag="vtr", name="vtr")
                        for i in range(TQ // 128):
                            nc.tensor.transpose(
                                ptr[:, i * 128 : (i + 1) * 128],
                                vt[:, i * 128 : (i + 1) * 128],
                                id_sb[:],
                            )
                        nc.scalar.copy(
                            vstd_all[:, lb * T + n * TQ : lb * T + (n + 1) * TQ],
                            ptr[:],
                        )
                # stream Wo^T in behind this unit's x tiles (2 chunks/unit)
                for jc in (2 * unit, 2 * unit + 1):
                    nc.sync.dma_start(
                        out=wot_sb[:, jc * C : (jc + 1) * C],
                        in_=wot_d[jc * 128 : (jc + 1) * 128, :],
                    )
                unit += 1


def _phase2_attn_wo(nc, tc, mask_sb, negid_sb, ones_sb, boc_sb, wot_sb,
                    q_all, kt_all, vstd_all, in_bufs, out_bufs, out_d):
    with (
        tc.tile_pool(name="pe", bufs=10) as pe,
        tc.tile_pool(name="pd", bufs=3) as pd,
        tc.tile_pool(name="pn", bufs=4) as pn,
        tc.tile_pool(name="pr", bufs=3) as pr,
        tc.tile_pool(name="pa", bufs=20) as pa,
        tc.tile_pool(name="po", bufs=4) as po,
        tc.tile_pool(name="pss", bufs=3, space="PSUM") as pss,
        tc.tile_pool(name="pso", bufs=1, space="PSUM") as pso,
    ):
        def attn_unit(lb, hl, qcg):
            qb = (hl * BL + lb) * T
            qcs = (2 * qcg, 2 * qcg + 1)
            kimax = [qc * 4 + 3 for qc in qcs]
            q_aps = [
                q_all[:, qb + qc * TQ : qb + (qc + 1) * TQ] for qc in qcs
            ]
            po_t = pso.tile([128, 2 * TQ], F32, tag="o", name="po")
            psum_o = [po_t[:, 0:TQ], po_t[:, TQ : 2 * TQ]]
            accs = [None, None]

            def emit_s(k0):
                # diagonal tiles: pre-initialize the PSUM region with the
                # causal mask via a PE matmul ((-1e30*I).T @ mask01), then
                # accumulate scores on top — keeps the Vector engine out of
                # the score->exp chain entirely
                exps = {}
                for kp in range(2):
                    klo = k0 + 2 * kp
                    for qi, qc in enumerate(qcs):
                        if klo > kimax[qi]:
                            continue
                        ps_s = pss.tile([128, 2 * TQ], F32, tag="s", name="pss")
                        for j in range(2):
                            ki = klo + j
                            di = ki - qc * 4
                            lo = di * 128 if di > 0 else 0
                            if di >= 0:
                                nc.tensor.matmul(
                                    ps_s[:, j * TQ : (j + 1) * TQ],
                                    negid_sb[:],
                                    mask_sb[:, di * TQ : (di + 1) * TQ],
                                    start=True,
                                    stop=False,
                                    skip_group_check=True,
                                )
                            nc.tensor.matmul(
                                ps_s[:, j * TQ + lo : (j + 1) * TQ],
                                kt_all[:, lb * T + ki * 128 :
                                       lb * T + (ki + 1) * 128],
                                q_aps[qi][:, lo:TQ],
                                start=(di < 0),
                                stop=True,
                                skip_group_check=True,
                            )
                        ex_sb = pe.tile([128, 2 * TQ], BF16, tag="e", name="ex")
                        nc.scalar.activation(
                            ex_sb[:],
                            ps_s[:],
                            mybir.ActivationFunctionType.Exp,
                            scale=SCALE,
                        )
                        exps[(qi, kp)] = ex_sb
                return exps

            def emit_pvd(k0, exps):
                for kp in range(2):
                    for j in range(2):
                        ki = k0 + 2 * kp + j
                        vsl = vstd_all[
                            :, lb * T + ki * 128 : lb * T + (ki + 1) * 128
                        ]
                        for qi in range(2):
                            if ki > kimax[qi] or (qi, kp) not in exps:
                                continue
                            nc.tensor.matmul(
                                psum_o[qi],
                                vsl,
                                exps[(qi, kp)][:, j * TQ : (j + 1) * TQ],
                                start=(ki == 0),
                                stop=(ki == kimax[qi]),
                            )
                # denominator fold tree on Vector only: the GpSimd queue
                # must stay empty, because the AllToAll triggers live there
                # and block in-order until the prior collective completes
                for qi in range(2):
                    folds = []
                    for kp in range(2):
                        if (qi, kp) not in exps:
                            continue
                        ex_sb = exps[(qi, kp)]
                        f = pd.tile([128, TQ], BF16, tag="f", name="f")
                        nc.vector.tensor_tensor(
                            f[:], ex_sb[:, 0:TQ], ex_sb[:, TQ : 2 * TQ],
                            AX.add,
                        )
                        folds.append(f)
                    if not folds:
                        continue
                    if len(folds) == 2:
                        cs_t = pd.tile([128, TQ], BF16, tag="cs", name="cs")
                        nc.vector.tensor_tensor(
                            cs_t[:], folds[0][:], folds[1][:], AX.add
                        )
                    else:
                        cs_t = folds[0]
                    if accs[qi] is None:
                        accs[qi] = cs_t
                    else:
                        na = pd.tile([128, TQ], BF16, tag=f"a{qi}", name="acc")
                        nc.vector.tensor_tensor(
                            na[:], accs[qi][:], cs_t[:], AX.add
                        )
                        accs[qi] = na

            pending = None
            for k0 in range(0, kimax[1] + 1, 4):
                exps = emit_s(k0)
                if pending is not None:
                    emit_pvd(*pending)
                pending = (k0, exps)
            emit_pvd(*pending)

            ps_df = pss.tile([128, 2 * TQ], F32, tag="s", name="pdf")
            for qi in range(2):
                nc.tensor.matmul(
                    ps_df[:, qi * TQ : (qi + 1) * TQ],
                    ones_sb[:], accs[qi][:],
                    start=True, stop=True,
                )
            for qi, qc in enumerate(qcs):
                rec = pr.tile([128, TQ], F32, tag="r", name="rec")
                nc.vector.reciprocal_approx_fast(
                    rec[:], ps_df[:, qi * TQ : (qi + 1) * TQ]
                )
                onrm = pn.tile([128, TQ], BF16, tag="on", name="onrm")
                nc.vector.tensor_tensor(
                    onrm[:], psum_o[qi], rec[:], AX.mult
                )
                h2, hr = hl // 2, hl % 2
                for th in range(2):
                    j = 2 * qc + th
                    nc.sync.dma_start(
                        out=in_bufs[lb][h2][
                            j * 256 + hr * 128 : j * 256 + (hr + 1) * 128, :
                        ],
                        in_=onrm[:, th * TW : (th + 1) * TW],
                    )

        def emit_a2a(lb, h2):
            nc.gpsimd.collective_compute(
                "AllToAll",
                AX.bypass,
                replica_groups=GROUPS,
                ins=[in_bufs[lb][h2].opt()],
                outs=[out_bufs[lb][h2].opt()],
            )

        # channel block cb (of 16) lives in half (cb%4)//2; order the wo
        # contraction half-0 blocks first so those matmuls only need the
        # first collective
        WO_JCS = [cb for cb in range(CCH) if (cb % 4) // 2 == 0] + [
            cb for cb in range(CCH) if (cb % 4) // 2 == 1
        ]

        def load_atts(lb):
            # cols 0:256 = batch lb, cols 256:512 = batch 2+lb (same tokens)
            atts = {}
            for cb in WO_JCS:
                j, half, hr = cb // 4, (cb % 4) // 2, cb % 2
                a = pa.tile([128, 2 * TW], BF16, tag="att", name="att")
                nc.sync.dma_start(
                    out=a[:, 0:TW],
                    in_=out_bufs[lb][half][
                        j * 256 + hr * 128 : j * 256 + (hr + 1) * 128, :
                    ],
                )
                nc.sync.dma_start(
                    out=a[:, TW : 2 * TW],
                    in_=out_bufs[lb][half][
                        1024 + j * 256 + hr * 128 : 1024 + j * 256 + (hr + 1) * 128,
                        :,
                    ],
                )
                atts[cb] = a
            return atts

        def wo_unit(lb, atts, cs):
            psum = pss.tile([128, 2 * TQ], F32, tag="s", name="pwo")
            for i, jc in enumerate(WO_JCS):
                nc.tensor.matmul(
                    psum[:, 0 : 2 * TW],
                    wot_sb[:, jc * C + cs * 128 : jc * C + (cs + 1) * 128],
                    atts[jc][:],
                    start=(i == 0),
                    stop=(i == CCH - 1),
                )
            osb = po.tile([128, 2 * TW], F32, tag="ou", name="osb")
            nc.scalar.activation(
                osb[:],
                psum[:, 0 : 2 * TW],
                mybir.ActivationFunctionType.Identity,
                bias=boc_sb[:, cs : cs + 1],
            )
            nc.sync.dma_start(
                out=out_d[
                    cs * 128 : (cs + 1) * 128, lb * 2 * TW : (lb + 1) * 2 * TW
                ],
                in_=osb[:],
            )

        # Sequential emission: the scheduler fills late attn(1) stalls with
        # wo(0) matmuls once the modeled collectives complete; each A2A half
        # fires as soon as its two heads' stores are done, and plenty of
        # attention PE work separates it from the first dependent wo matmul
        # in the in-order PE queue.  (Interleaving wo units into attn
        # emission puts A2A-dependent matmuls too early and stalls the PE.)
        for hl in range(HPC):
            for qcg in range(2):
                attn_unit(0, hl, qcg)
            if hl == 1:
                emit_a2a(0, 0)
        emit_a2a(0, 1)
        for hl in range(HPC):
            for qcg in range(2):
                attn_unit(1, hl, qcg)
            if hl == 1:
                emit_a2a(1, 0)
        emit_a2a(1, 1)
        atts0 = load_atts(0)
        for cs in range(CCH):
            wo_unit(0, atts0, cs)
        atts1 = load_atts(1)
        for cs in range(CCH):
            wo_unit(1, atts1, cs)


def _prep_inputs(x, rope_cos, rope_sin, Wq, Wkv, Wo, bo):
    x = np.asarray(x, np.float32)
    rope_cos = np.asarray(rope_cos, np.float32)
    rope_sin = np.asarray(rope_sin, np.float32)
    Wq = np.asarray(Wq, np.float32)
    Wkv = np.asarray(Wkv, np.float32)
    Wo = np.asarray(Wo, np.float32)
    bo = np.asarray(bo, np.float32)

    xt = np.ascontiguousarray(x.transpose(0, 2, 1)).astype(bfloat16)  # (B, C, T)
    wot = np.ascontiguousarray(Wo.T).astype(bfloat16)                 # (j, c_out)
    cc = np.concatenate([rope_cos.T, rope_cos.T], axis=0).astype(bfloat16)
    ss = np.concatenate([-rope_sin.T, rope_sin.T], axis=0).astype(bfloat16)

    # 0/1 mask pattern; the kernel turns it into -1e30 adds on the PE via
    # (negid.T @ mask01) PSUM pre-initialization
    masks = np.zeros((128, 4 * TQ), np.float32)
    kp = np.arange(128)[:, None]
    qf = np.arange(TQ)[None, :]
    for di in range(4):
        masks[:, di * TQ : (di + 1) * TQ] = np.where(kp + di * 128 <= qf, 0.0, 1.0)
    masks = masks.astype(bfloat16)
    negid = (np.eye(128, dtype=np.float32) * -1e30).astype(bfloat16)

    ones = np.ones((128, 128), bfloat16)
    ident = np.eye(128, dtype=np.float32).astype(bfloat16)
    boc = np.ascontiguousarray(bo.reshape(CCH, 128).T)  # [p, cs]

    in_maps = []
    for c in range(NCORES):
        h, g = c // 4, c % 4
        wqkv = np.ascontiguousarray(
            np.concatenate(
                [Wq[(4 * g + m) * HD : (4 * g + m + 1) * HD, :].T
                 for m in range(4)]
                + [
                    Wkv[g * HD : (g + 1) * HD, :].T,
                    Wkv[N_KV * HD + g * HD : N_KV * HD + (g + 1) * HD, :].T,
                ],
                axis=1,
            )
        ).astype(bfloat16)
        in_maps.append(
            {
                "xt": np.ascontiguousarray(xt[2 * h : 2 * h + 2]),
                "wqkv": wqkv,
                "wot": wot,
                "ropec": cc,
                "ropes": ss,
                "masks": masks,
                "negid": negid,
                "ones": ones,
                "ident": ident,
                "boc": boc,
            }
        )
    return in_maps


def kernel(x, rope_cos, rope_sin, Wq, Wkv, Wo, bo):
    if "nc" not in _CACHE:
        _CACHE["nc"] = _build()
    nc = _CACHE["nc"]
    in_maps = _prep_inputs(x, rope_cos, rope_sin, Wq, Wkv, Wo, bo)

    trace = bool(int(os.environ.get("KERNEL_TRACE", "0")))
    kw = {}
    if trace:
        _install_trace_hook()
        kw["trace"] = True
    res = run_bass_kernel_spmd(nc, in_maps, core_ids=list(range(NCORES)), **kw)
    _CACHE["exec_time_ns"] = res.exec_time_ns

    # per-core out is [C, BL*2*TW]: token slice [c*TW:(c+1)*TW] of batches
    # (lb, 2+lb) packed per lb; reassemble
    out = np.empty((B, T, C), np.float32)
    for c in range(NCORES):
        o = res.results[c]["out"]  # (C, 1024)
        for lb in range(BL):
            out[lb, c * TW : (c + 1) * TW, :] = o[
                :, lb * 2 * TW : lb * 2 * TW + TW
            ].T
            out[2 + lb, c * TW : (c + 1) * TW, :] = o[
                :, lb * 2 * TW + TW : (lb + 1) * 2 * TW
            ].T
    return out


def _install_trace_hook():
    """Register the NTFF profiling hook (missing antenv.axon_hooks shim)."""
    import types

    import antenv
    from concourse import bass_utils

    if not hasattr(antenv, "axon_hooks"):
        mod = types.ModuleType("antenv.axon_hooks")
        hook = [None]
        mod.set_axon_ntff_profile_hook = lambda h: hook.__setitem__(0, h)
        mod.get_axon_ntff_profile_hook = lambda: hook[0]
        sys.modules["antenv.axon_hooks"] = mod
        antenv.axon_hooks = mod
        try:
            from trn_agent_boot.trn_boot import _ntff_profile_via_ctypes

            mod.set_axon_ntff_profile_hook(
                _ntff_profile_via_ctypes("/opt/axon/libaxon_pjrt.so")
            )
        except Exception:
            pass
    bass_utils.upload_artifacts = lambda tmpdir: f"local://{tmpdir}"


# revision 50
# speedup vs baseline: 1.0174x; 1.0174x over previous
"""Trainium2 Bass kernel: decoder GQA attention with RoPE, tensor-parallel over 8 NeuronCores.

Sharding: core c = (h, g) with h = c//4, g = c%4 handles the 4 query heads of
GQA group g (heads 4g..4g+3, which share KV head g) for the 2 batches
{2h, 2h+1}.  Compared with 2-heads x 4-batches per core this removes the
duplicated K/V projections entirely (48 instead of 64 projection matmuls per
token chunk) without any extra collective, halves the x DMA, and widens the
output-projection matmuls to 512 moving columns.  All matmul operands are
bf16 (same PE rate as fp32r, half the DMA/SBUF traffic); PSUM and softmax
denominators stay fp32.  Per core:
  - Constants (RoPE tables, causal masks, Wo^T, bias) are DMA'd at kernel
    start so the attention phase never waits on them; Wo^T chunks are
    interleaved with the projection units to stay off the x-stream's critical
    path.
  - QKV projection of the core's 2 batches against its [C, 768] weight slice,
    emitted output-major (q01 | q23 | kv accumulation groups) so each PSUM
    ring slot drains while the next group computes; RoPE on the fly; q/k/v
    stay SBUF-resident.
  - Flash-style causal attention with transposed scores (sT[k,q]) in
    [128,1024] PSUM tiles, exp batched per 1024 cols on the Scalar engine,
    software-pipelined so PV matmuls of the previous key-chunk fill the PE
    while the current chunk exponentiates.  The causal mask is applied by
    the PE itself: diagonal tiles' PSUM is pre-initialized with
    (-1e30*I).T @ mask01 and the (diagonal-trimmed) scores accumulate on
    top, so the Vector engine stays out of the score->exp chain.  The
    softmax denominator is a bf16 fold-tree on the Vector engine plus one
    ones-matmul per query chunk; normalization uses the fast approximate
    reciprocal.
  - One 8-core AllToAll per (local batch, head-half) reshards the attention
    output head->token in 256-token chunks (cores 0-3 contribute batch lb,
    cores 4-7 batch 2+lb, so every core ends with both batches' full
    channels for its token slice and the output projection keeps 512 moving
    columns).  Each half-collective fires as soon as its two heads' stores
    land; a full attention batch of PE work separates it from the first
    dependent wo matmul in the in-order PE queue, hiding the collective.
    The GpSimd queue carries the collective triggers (which block in-order
    until their collective completes) plus only the phase-1 x-tile DMAs;
    the ring-warmup collective is emitted after phase 1 for the same
    reason.  Wo bias is fused into the Scalar PSUM->SBUF copy; the host
    transposes at assembly.
"""

import os
import sys

for _p in ("/opt/trn_rl_repo",):
    if _p not in sys.path:
        sys.path.insert(0, _p)

import numpy as np
from ml_dtypes import bfloat16

import concourse.bacc as bacc
import concourse.mybir as mybir
import concourse.tile as tile
from concourse.bass_utils import run_bass_kernel_spmd

F32 = mybir.dt.float32
BF16 = mybir.dt.bfloat16
AX = mybir.AluOpType

B, T, C = 4, 2048, 2048
N_HEAD, N_KV = 16, 4
HD = C // N_HEAD            # 128
NCORES = 8
GROUPS = [list(range(NCORES))]
HPC = 4                     # q heads per core (one full GQA group)
BL = 2                      # local batches per core
SCALE = 1.0 / float(np.sqrt(HD))
TQ = 512                    # query-chunk (psum free dim)
NQC = T // TQ               # 4 query chunks per (lb, head)
CCH = C // 128              # 16 contraction chunks
TW = T // NCORES            # 256 tokens per (core, batch) in the wo shard

_CACHE = {}


def _build():
    """Build + compile the per-core Bass graph (same graph for every core)."""
    nc = bacc.Bacc(
        "TRN2",
        target_bir_lowering=False,
        debug=False,
        enable_asserts=False,
        num_devices=NCORES,
    )

    xt_d = nc.dram_tensor("xt", [BL, C, T], BF16, kind="ExternalInput")
    wqkv_d = nc.dram_tensor("wqkv", [C, 768], BF16, kind="ExternalInput")
    wot_d = nc.dram_tensor("wot", [C, C], BF16, kind="ExternalInput")
    cc_d = nc.dram_tensor("ropec", [128, T], BF16, kind="ExternalInput")
    ss_d = nc.dram_tensor("ropes", [128, T], BF16, kind="ExternalInput")
    mask_d = nc.dram_tensor("masks", [128, 4 * TQ], BF16, kind="ExternalInput")
    negid_d = nc.dram_tensor("negid", [128, 128], BF16, kind="ExternalInput")
    ones_d = nc.dram_tensor("ones", [128, 128], BF16, kind="ExternalInput")
    ident_d = nc.dram_tensor("ident", [128, 128], BF16, kind="ExternalInput")
    boc_d = nc.dram_tensor("boc", [128, CCH], F32, kind="ExternalInput")
    out_d = nc.dram_tensor("out", [C, BL * 2 * TW], F32, kind="ExternalOutput")

    with tile.TileContext(nc) as tc:
        with tc.tile_pool(name="dram", bufs=1, space="DRAM") as dp:
            # one buffer per (local batch, head-half): halved collectives
            # start earlier and the scheduler's collective-cost model then
            # lets wo matmuls fill attention stalls sooner
            in_bufs = [
                [dp.tile([C, TW], BF16, name=f"in_buf{b}_{h2}") for h2 in range(2)]
                for b in range(BL)
            ]
            out_bufs = [
                [dp.tile([C, TW], BF16, name=f"out_buf{b}_{h2}") for h2 in range(2)]
                for b in range(BL)
            ]
            warm_in = dp.tile([NCORES, 64], BF16, name="warm_in")
            warm_out = dp.tile([NCORES, 64], BF16, name="warm_out")
            with tc.tile_pool(name="res", bufs=1) as rp:
                # constants resident for the whole kernel; DMAs are emitted by
                # phase 1 after the first unit's w/x loads so the first
                # matmuls are not queued behind them
                id_sb = rp.tile([128, 128], BF16, name="id_sb")
                cc_sb = rp.tile([128, T], BF16, name="cc_sb")
                ss_sb = rp.tile([128, T], BF16, name="ss_sb")
                mask_sb = rp.tile([128, 4 * TQ], BF16, name="mask_sb")
                negid_sb = rp.tile([128, 128], BF16, name="negid_sb")
                ones_sb = rp.tile([128, 128], BF16, name="ones_sb")
                boc_sb = rp.tile([128, CCH], F32, name="boc_sb")
                wot_sb = rp.tile([128, CCH * C], BF16, name="wot_sb")

                def emit_consts():
                    nc.sync.dma_start(out=id_sb[:], in_=ident_d.ap())
                    nc.sync.dma_start(out=cc_sb[:], in_=cc_d.ap())
                    nc.sync.dma_start(out=ss_sb[:], in_=ss_d.ap())
                    nc.sync.dma_start(out=mask_sb[:], in_=mask_d.ap())
                    nc.sync.dma_start(out=negid_sb[:], in_=negid_d.ap())
                    nc.sync.dma_start(out=ones_sb[:], in_=ones_d.ap())
                    nc.sync.dma_start(out=boc_sb[:], in_=boc_d.ap())

                kt_all = rp.tile([128, BL * T], BF16, name="kt_all")
                vstd_all = rp.tile([128, BL * T], BF16, name="vstd_all")
                q_all = rp.tile([128, HPC * BL * T], BF16, name="q_all")

                _phase1_qkv(nc, tc, xt_d, wqkv_d, wot_d, id_sb, cc_sb, ss_sb,
                            wot_sb, q_all, kt_all, vstd_all, emit_consts)
                # warm the collective rings after phase 1: the trigger blocks
                # the GpSimd queue until the collective completes, and the
                # phase-1 x-tile DMAs live on that queue.  Writing warm_in
                # from late phase-1 output gives the trigger a real data
                # dependency so the scheduler cannot hoist it into phase 1.
                nc.sync.dma_start(
                    out=warm_in[:],
                    in_=q_all[0:8, HPC * BL * T - 64 : HPC * BL * T],
                )
                nc.gpsimd.collective_compute(
                    "AllToAll",
                    AX.bypass,
                    replica_groups=GROUPS,
                    ins=[warm_in.opt()],
                    outs=[warm_out.opt()],
                )
                _phase2_attn_wo(nc, tc, mask_sb, negid_sb, ones_sb, boc_sb,
                                wot_sb, q_all, kt_all, vstd_all, in_bufs,
                                out_bufs, out_d)

    nc.compile()
    return nc


def _phase1_qkv(nc, tc, xt_d, wqkv_d, wot_d, id_sb, cc_sb, ss_sb, wot_sb,
                q_all, kt_all, vstd_all, emit_consts):
    with (
        tc.tile_pool(name="p1c", bufs=1) as p1c,
        tc.tile_pool(name="px", bufs=32) as px,
        tc.tile_pool(name="pt", bufs=3) as pt,
        tc.tile_pool(name="pp", bufs=3, space="PSUM") as pp,
        tc.tile_pool(name="pst", bufs=2, space="PSUM") as pst,
    ):
        # first unit's weights (Sync queue) and x tiles (Scalar queue) issue
        # in parallel so the first ci-major matmul group starts after ~2
        # DMAs.  x tiles stay off the GpSimd queue: the collective triggers
        # live there and block in-order until their collective completes.
        w_sb = p1c.tile([128, CCH * 768], BF16, name="w_sb")
        warm_xts = []
        for ci in range(CCH):
            nc.sync.dma_start(
                out=w_sb[:, ci * 768 : (ci + 1) * 768],
                in_=wqkv_d[ci * 128 : (ci + 1) * 128, :],
            )
            xtile = px.tile([128, TQ], BF16, tag="xt", name="xt")
            nc.gpsimd.dma_start(
                out=xtile[:], in_=xt_d[0, ci * 128 : (ci + 1) * 128, 0:TQ]
            )
            warm_xts.append(xtile)
        emit_consts()

        def rope(psrc, dst_ap, cs):
            # dst = src*cc + swap_halves(src)*ss   (rotate-half RoPE)
            qs = pt.tile([128, TQ], BF16, tag="qs", name="qs")
            nc.scalar.copy(qs[:], psrc)
            qsw = pt.tile([128, TQ], BF16, tag="qsw", name="qsw")
            nc.scalar.dma_start(out=qsw[0:64, :], in_=qs[64:128, :])
            nc.scalar.dma_start(out=qsw[64:128, :], in_=qs[0:64, :])
            tm1 = pt.tile([128, TQ], BF16, tag="tm1", name="tm1")
            nc.vector.tensor_tensor(tm1[:], qs[:], cc_sb[:, cs], AX.mult)
            tm2 = pt.tile([128, TQ], BF16, tag="tm2", name="tm2")
            nc.vector.tensor_tensor(tm2[:], qsw[:], ss_sb[:, cs], AX.mult)
            nc.vector.tensor_tensor(dst_ap, tm1[:], tm2[:], AX.add)

        unit = 0
        for lb in range(BL):
            for n in range(NQC):
                if lb == 0 and n == 0:
                    xts = warm_xts
                else:
                    xts = []
                    for ci in range(CCH):
                        xtile = px.tile([128, TQ], BF16, tag="xt", name="xt")
                        nc.gpsimd.dma_start(
                            out=xtile[:],
                            in_=xt_d[
                                lb, ci * 128 : (ci + 1) * 128, n * TQ : (n + 1) * TQ
                            ],
                        )
                        xts.append(xtile)
                cs = slice(n * TQ, (n + 1) * TQ)
                # output-major accumulation groups: q01 | q23 | kv, so each
                # PSUM ring slot is drained while the next group computes.
                # The first unit is ci-major instead: its matmuls then only
                # wait for w[0]/x[0] rather than the full weight+x load.
                ps = [
                    pp.tile([128, 2 * TQ], F32, tag="proj", name=f"ps{i}")
                    for i in range(3)
                ]
                if unit == 0:
                    for ci in range(CCH):
                        for m in range(6):
                            nc.tensor.matmul(
                                ps[m // 2][:, (m % 2) * TQ : (m % 2 + 1) * TQ],
                                w_sb[:, ci * 768 + m * 128 : ci * 768 + (m + 1) * 128],
                                xts[ci][:],
                                start=(ci == 0),
                                stop=(ci == CCH - 1),
                            )
                for grp in range(3):
                    if unit > 0:
                        for ci in range(CCH):
                            for half in range(2):
                                m = 2 * grp + half
                                nc.tensor.matmul(
                                    ps[grp][:, half * TQ : (half + 1) * TQ],
                                    w_sb[:, ci * 768 + m * 128 : ci * 768 + (m + 1) * 128],
                                    xts[ci][:],
                                    start=(ci == 0),
                                    stop=(ci == CCH - 1),
                                )
                    if grp < 2:
                        for half in range(2):
                            hl = 2 * grp + half
                            rope(ps[grp][:, half * TQ : (half + 1) * TQ],
                                 q_all[:, (hl * BL + lb) * T + n * TQ :
                                       (hl * BL + lb) * T + (n + 1) * TQ], cs)
                    else:
                        rope(ps[grp][:, 0:TQ],
                             kt_all[:, lb * T + n * TQ : lb * T + (n + 1) * TQ],
                             cs)
                        vt = pt.tile([128, TQ], BF16, tag="vt", name="vt")
                        nc.scalar.copy(vt[:], ps[grp][:, TQ : 2 * TQ])
                        ptr = pst.tile([128, TQ], BF16, tag="vtr", name="vtr")
                        for i in range(TQ // 128):
                            nc.tensor.transpose(
                                ptr[:, i * 128 : (i + 1) * 128],
                                vt[:, i * 128 : (i + 1) * 128],
                                id_sb[:],
                            )
                        nc.scalar.copy(
                            vstd_all[:, lb * T + n * TQ : lb * T + (n + 1) * TQ],
                            ptr[:],
                        )
                # stream Wo^T in behind this unit's x tiles (2 chunks/unit)
                for jc in (2 * unit, 2 * unit + 1):
                    nc.sync.dma_start(
                        out=wot_sb[:, jc * C : (jc + 1) * C],
                        in_=wot_d[jc * 128 : (jc + 1) * 128, :],
                    )
                unit += 1


def _phase2_attn_wo(nc, tc, mask_sb, negid_sb, ones_sb, boc_sb, wot_sb,
                    q_all, kt_all, vstd_all, in_bufs, out_bufs, out_d):
    with (
        tc.tile_pool(name="pe", bufs=10) as pe,
        tc.tile_pool(name="pd", bufs=3) as pd,
        tc.tile_pool(name="pn", bufs=4) as pn,
        tc.tile_pool(name="pr", bufs=3) as pr,
        tc.tile_pool(name="pa", bufs=20) as pa,
        tc.tile_pool(name="po", bufs=4) as po,
        tc.tile_pool(name="pss", bufs=3, space="PSUM") as pss,
        tc.tile_pool(name="pso", bufs=1, space="PSUM") as pso,
    ):
        def attn_unit(lb, hl, qcg):
            qb = (hl * BL + lb) * T
            qcs = (2 * qcg, 2 * qcg + 1)
            kimax = [qc * 4 + 3 for qc in qcs]
            q_aps = [
                q_all[:, qb + qc * TQ : qb + (qc + 1) * TQ] for qc in qcs
            ]
            po_t = pso.tile([128, 2 * TQ], F32, tag="o", name="po")
            psum_o = [po_t[:, 0:TQ], po_t[:, TQ : 2 * TQ]]
            accs = [None, None]

            def emit_s(k0):
                # diagonal tiles: pre-initialize the PSUM region with the
                # causal mask via a PE matmul ((-1e30*I).T @ mask01), then
                # accumulate scores on top — keeps the Vector engine out of
                # the score->exp chain entirely
                exps = {}
                for kp in range(2):
                    klo = k0 + 2 * kp
                    for qi, qc in enumerate(qcs):
                        if klo > kimax[qi]:
                            continue
                        ps_s = pss.tile([128, 2 * TQ], F32, tag="s", name="pss")
                        for j in range(2):
                            ki = klo + j
                            di = ki - qc * 4
                            lo = di * 128 if di > 0 else 0
                            if di >= 0:
                                nc.tensor.matmul(
                                    ps_s[:, j * TQ : (j + 1) * TQ],
                                    negid_sb[:],
                                    mask_sb[:, di * TQ : (di + 1) * TQ],
                                    start=True,
                                    stop=False,
                                    skip_group_check=True,
                                )
                            nc.tensor.matmul(
                                ps_s[:, j * TQ + lo : (j + 1) * TQ],
                                kt_all[:, lb * T + ki * 128 :
                                       lb * T + (ki + 1) * 128],
                                q_aps[qi][:, lo:TQ],
                                start=(di < 0),
                                stop=True,
                                skip_group_check=True,
                            )
                        ex_sb = pe.tile([128, 2 * TQ], BF16, tag="e", name="ex")
                        nc.scalar.activation(
                            ex_sb[:],
                            ps_s[:],
                            mybir.ActivationFunctionType.Exp,
                            scale=SCALE,
                        )
                        exps[(qi, kp)] = ex_sb
                return exps

            def emit_pvd(k0, exps):
                for kp in range(2):
                    for j in range(2):
                        ki = k0 + 2 * kp + j
                        vsl = vstd_all[
                            :, lb * T + ki * 128 : lb * T + (ki + 1) * 128
                        ]
                        for qi in range(2):
                            if ki > kimax[qi] or (qi, kp) not in exps:
                                continue
                            nc.tensor.matmul(
                                psum_o[qi],
                                vsl,
                                exps[(qi, kp)][:, j * TQ : (j + 1) * TQ],
                                start=(ki == 0),
                                stop=(ki == kimax[qi]),
                            )
                # denominator fold tree on Vector only: the GpSimd queue
                # must stay empty, because the AllToAll triggers live there
                # and block in-order until the prior collective completes
                for qi in range(2):
                    folds = []
                    for kp in range(2):
                        if (qi, kp) not in exps:
                            continue
                        ex_sb = exps[(qi, kp)]
                        f = pd.tile([128, TQ], BF16, tag="f", name="f")
                        nc.vector.tensor_tensor(
                            f[:], ex_sb[:, 0:TQ], ex_sb[:, TQ : 2 * TQ],
                            AX.add,
                        )
                        folds.append(f)
                    if not folds:
                        continue
                    if len(folds) == 2:
                        cs_t = pd.tile([128, TQ], BF16, tag="cs", name="cs")
                        nc.vector.tensor_tensor(
                            cs_t[:], folds[0][:], folds[1][:], AX.add
                        )
                    else:
                        cs_t = folds[0]
                    if accs[qi] is None:
                        accs[qi] = cs_t
                    else:
                        na = pd.tile([128, TQ], BF16, tag=f"a{qi}", name="acc")
                        nc.vector.tensor_tensor(
                            na[:], accs[qi][:], cs_t[:], AX.add
                        )
                        accs[qi] = na

            pending = None
            for k0 in range(0, kimax[1] + 1, 4):
                exps = emit_s(k0)
                if pending is not None:
                    emit_pvd(*pending)
                pending = (k0, exps)
            emit_pvd(*pending)

            ps_df = pss.tile([128, 2 * TQ], F32, tag="s", name="pdf")
            for qi in range(2):
                nc.tensor.matmul(
                    ps_df[:, qi * TQ : (qi + 1) * TQ],
                    ones_sb[:], accs[qi][:],
                    start=True, stop=True,
                )
            for qi, qc in enumerate(qcs):
                rec = pr.tile([128, TQ], F32, tag="r", name="rec")
                nc.vector.reciprocal_approx_fast(
                    rec[:], ps_df[:, qi * TQ : (qi + 1) * TQ]
                )
                onrm = pn.tile([128, TQ], BF16, tag="on", name="onrm")
                nc.vector.tensor_tensor(
                    onrm[:], psum_o[qi], rec[:], AX.mult
                )
                h2, hr = hl // 2, hl % 2
                for th in range(2):
                    j = 2 * qc + th
                    nc.sync.dma_start(
                        out=in_bufs[lb][h2][
                            j * 256 + hr * 128 : j * 256 + (hr + 1) * 128, :
                        ],
                        in_=onrm[:, th * TW : (th + 1) * TW],
                    )

        def emit_a2a(lb, h2):
            nc.gpsimd.collective_compute(
                "AllToAll",
                AX.bypass,
                replica_groups=GROUPS,
                ins=[in_bufs[lb][h2].opt()],
                outs=[out_bufs[lb][h2].opt()],
            )

        # channel block cb (of 16) lives in half (cb%4)//2; order the wo
        # contraction half-0 blocks first so those matmuls only need the
        # first collective
        WO_JCS = [cb for cb in range(CCH) if (cb % 4) // 2 == 0] + [
            cb for cb in range(CCH) if (cb % 4) // 2 == 1
        ]

        def load_atts(lb):
            # cols 0:256 = batch lb, cols 256:512 = batch 2+lb (same tokens)
            atts = {}
            for cb in WO_JCS:
                j, half, hr = cb // 4, (cb % 4) // 2, cb % 2
                a = pa.tile([128, 2 * TW], BF16, tag="att", name="att")
                nc.sync.dma_start(
                    out=a[:, 0:TW],
                    in_=out_bufs[lb][half][
                        j * 256 + hr * 128 : j * 256 + (hr + 1) * 128, :
                    ],
                )
                nc.sync.dma_start(
                    out=a[:, TW : 2 * TW],
                    in_=out_bufs[lb][half][
                        1024 + j * 256 + hr * 128 : 1024 + j * 256 + (hr + 1) * 128,
                        :,
                    ],
                )
                atts[cb] = a
            return atts

        def wo_unit(lb, atts, cs):
            psum = pss.tile([128, 2 * TQ], F32, tag="s", name="pwo")
            for i, jc in enumerate(WO_JCS):
                nc.tensor.matmul(
                    psum[:, 0 : 2 * TW],
                    wot_sb[:, jc * C + cs * 128 : jc * C + (cs + 1) * 128],
                    atts[jc][:],
                    start=(i == 0),
                    stop=(i == CCH - 1),
                )
            osb = po.tile([128, 2 * TW], F32, tag="ou", name="osb")
            nc.scalar.activation(
                osb[:],
                psum[:, 0 : 2 * TW],
                mybir.ActivationFunctionType.Identity,
                bias=boc_sb[:, cs : cs + 1],
            )
            nc.sync.dma_start(
                out=out_d[
                    cs * 128 : (cs + 1) * 128, lb * 2 * TW : (lb + 1) * 2 * TW
                ],
                in_=osb[:],
            )

        # Sequential emission: the scheduler fills late attn(1) stalls with
        # wo(0) matmuls once the modeled collectives complete; each A2A half
        # fires as soon as its two heads' stores are done, and plenty of
        # attention PE work separates it from the first dependent wo matmul
        # in the in-order PE queue.  (Interleaving wo units into attn
        # emission puts A2A-dependent matmuls too early and stalls the PE.)
        for hl in range(HPC):
            for qcg in range(2):
                attn_unit(0, hl, qcg)
            if hl == 1:
                emit_a2a(0, 0)
        emit_a2a(0, 1)
        for hl in range(HPC):
            for qcg in range(2):
                attn_unit(1, hl, qcg)
            if hl == 1:
                emit_a2a(1, 0)
        emit_a2a(1, 1)
        atts0 = load_atts(0)
        for cs in range(CCH):
            wo_unit(0, atts0, cs)
        atts1 = load_atts(1)
        for cs in range(CCH):
            wo_unit(1, atts1, cs)


def _prep_inputs(x, rope_cos, rope_sin, Wq, Wkv, Wo, bo):
    x = np.asarray(x, np.float32)
    rope_cos = np.asarray(rope_cos, np.float32)
    rope_sin = np.asarray(rope_sin, np.float32)
    Wq = np.asarray(Wq, np.float32)
    Wkv = np.asarray(Wkv, np.float32)
    Wo = np.asarray(Wo, np.float32)
    bo = np.asarray(bo, np.float32)

    xt = np.ascontiguousarray(x.transpose(0, 2, 1)).astype(bfloat16)  # (B, C, T)
    wot = np.ascontiguousarray(Wo.T).astype(bfloat16)                 # (j, c_out)
    cc = np.concatenate([rope_cos.T, rope_cos.T], axis=0).astype(bfloat16)
    ss = np.concatenate([-rope_sin.T, rope_sin.T], axis=0).astype(bfloat16)

    # 0/1 mask pattern; the kernel turns it into -1e30 adds on the PE via
    # (negid.T @ mask01) PSUM pre-initialization
    masks = np.zeros((128, 4 * TQ), np.float32)
    kp = np.arange(128)[:, None]
    qf = np.arange(TQ)[None, :]
    for di in range(4):
        masks[:, di * TQ : (di + 1) * TQ] = np.where(kp + di * 128 <= qf, 0.0, 1.0)
    masks = masks.astype(bfloat16)
    negid = (np.eye(128, dtype=np.float32) * -1e30).astype(bfloat16)

    ones = np.ones((128, 128), bfloat16)
    ident = np.eye(128, dtype=np.float32).astype(bfloat16)
    boc = np.ascontiguousarray(bo.reshape(CCH, 128).T)  # [p, cs]

    in_maps = []
    for c in range(NCORES):
        h, g = c // 4, c % 4
        wqkv = np.ascontiguousarray(
            np.concatenate(
                [Wq[(4 * g + m) * HD : (4 * g + m + 1) * HD, :].T
                 for m in range(4)]
                + [
                    Wkv[g * HD : (g + 1) * HD, :].T,
                    Wkv[N_KV * HD + g * HD : N_KV * HD + (g + 1) * HD, :].T,
                ],
                axis=1,
            )
        ).astype(bfloat16)
        in_maps.append(
            {
                "xt": np.ascontiguousarray(xt[2 * h : 2 * h + 2]),
                "wqkv": wqkv,
                "wot": wot,
                "ropec": cc,
                "ropes": ss,
                "masks": masks,
                "negid": negid,
                "ones": ones,
                "ident": ident,
                "boc": boc,
            }
        )
    return in_maps


def kernel(x, rope_cos, rope_sin, Wq, Wkv, Wo, bo):
    if "nc" not in _CACHE:
        _CACHE["nc"] = _build()
    nc = _CACHE["nc"]
    in_maps = _prep_inputs(x, rope_cos, rope_sin, Wq, Wkv, Wo, bo)

    trace = bool(int(os.environ.get("KERNEL_TRACE", "0")))
    kw = {}
    if trace:
        _install_trace_hook()
        kw["trace"] = True
    res = run_bass_kernel_spmd(nc, in_maps, core_ids=list(range(NCORES)), **kw)
    _CACHE["exec_time_ns"] = res.exec_time_ns

    # per-core out is [C, BL*2*TW]: token slice [c*TW:(c+1)*TW] of batches
    # (lb, 2+lb) packed per lb; reassemble
    out = np.empty((B, T, C), np.float32)
    for c in range(NCORES):
        o = res.results[c]["out"]  # (C, 1024)
        for lb in range(BL):
            out[lb, c * TW : (c + 1) * TW, :] = o[
                :, lb * 2 * TW : lb * 2 * TW + TW
            ].T
            out[2 + lb, c * TW : (c + 1) * TW, :] = o[
                :, lb * 2 * TW + TW : (lb + 1) * 2 * TW
            ].T
    return out


def _install_trace_hook():
    """Register the NTFF profiling hook (missing antenv.axon_hooks shim)."""
    import types

    import antenv
    from concourse import bass_utils

    if not hasattr(antenv, "axon_hooks"):
        mod = types.ModuleType("antenv.axon_hooks")
        hook = [None]
        mod.set_axon_ntff_profile_hook = lambda h: hook.__setitem__(0, h)
        mod.get_axon_ntff_profile_hook = lambda: hook[0]
        sys.modules["antenv.axon_hooks"] = mod
        antenv.axon_hooks = mod
        try:
            from trn_agent_boot.trn_boot import _ntff_profile_via_ctypes

            mod.set_axon_ntff_profile_hook(
                _ntff_profile_via_ctypes("/opt/axon/libaxon_pjrt.so")
            )
        except Exception:
            pass
    bass_utils.upload_artifacts = lambda tmpdir: f"local://{tmpdir}"
